# revision 1
# baseline (speedup 1.0000x reference)
"""Trainium2 Bass kernel: Conv2d [8,8,1024,1024] x [8,8,3,3] (+bias), with
the reference's roll-by-1 on H, VALID padding -> [8,8,1022,1022].

Strategy: data-parallel over the batch dim (1 image per NeuronCore, 8 cores).
Per core the conv runs as blocked matmuls on the tensor engine:
  - SBUF input tile [128, W]: partition p = q*8 + cin holds (rolled) input row
    s = 14*b + q of channel cin (16 rows x 8 cin = 128 partitions).
  - lhsT [128, 112]: zero-padded weights; column m = dx*8 + co holds
    filt[co, cin, q-dx, j] at partition (q, cin) when 0 <= q-dx <= 2.
    M packs 14 output rows x 8 couts (dx-major: the output DMA's outer
    HBM dim is then dx=14, fanning across 14 SDMA engines).
  - The 3 W-taps (j) are 3 accumulating matmuls whose rhs is the same tile
    shifted by j in the free dim. dtype float32r (fast fp32 PE path).
  - PSUM [112, 511] is evicted by ScalarE activation(Identity, bias) into
    SBUF, then DMA'd out. The H-roll is folded into the input DMA rows.
"""

import os
import sys

for _p in ("/opt/trn_rl_repo",):
    if _p not in sys.path and os.path.isdir(_p):
        sys.path.insert(0, _p)

import numpy as np

import concourse.bacc as bacc
import concourse.bass as bass
import concourse.mybir as mybir
from concourse.bass_utils import run_bass_kernel_spmd
from concourse.tile import TileContext

F32 = mybir.dt.float32
F32R = mybir.dt.float32r

N_CORES = 8
CIN = 8
COUT = 8
KH = 3
KW = 3


def _pad32(n):
    return (n + 31) // 32 * 32


def _const_layout(D, d_last):
    """Column offsets in the packed consts array."""
    M = COUT * D
    off_bias = KW * M
    cw = off_bias + 1
    off_wl = off_bl = None
    if d_last:
        Ml = COUT * d_last
        off_wl = cw
        off_bl = off_wl + KW * Ml
        cw = off_bl + 1
    return dict(M=M, off_bias=off_bias, off_wl=off_wl, off_bl=off_bl,
                cw=cw, cw_pad=_pad32(cw))


def build_nc(
    H: int = 1024,
    W: int = 1024,
    D: int = 14,
    in_bufs: int = 16,
    out_bufs: int = 8,
    psum_bufs: int = 8,
):
    """Build the per-core Bass program. Returns (nc, meta)."""
    HOUT = H - (KH - 1)
    WOUT = W - (KW - 1)
    R = D + 2  # input rows resident per block
    assert R * CIN <= 128
    n_full = HOUT // D
    d_last = HOUT - n_full * D  # partial last block (0 for 1022/14)
    lay = _const_layout(D, d_last)
    M = lay["M"]
    # W chunks of <= 512, even (fp32r needs even N and wants N >= 256;
    # PSUM bank holds 512 fp32)
    assert WOUT % 2 == 0
    chunks = []
    w0 = 0
    while w0 < WOUT:
        n = min(512, WOUT - w0)
        if n % 2:
            n -= 1
        chunks.append((w0, n))
        w0 += n

    nc = bacc.Bacc("TRN2", target_bir_lowering=False, debug=False,
                   num_devices=N_CORES)
    inp_d = nc.dram_tensor("inp", [CIN, H, W], F32R, kind="ExternalInput")
    consts_d = nc.dram_tensor("consts", [128, lay["cw_pad"]], F32R,
                              kind="ExternalInput")
    out_d = nc.dram_tensor("out", [COUT, HOUT, WOUT], F32, kind="ExternalOutput")

    ident = mybir.ActivationFunctionType.Identity

    with TileContext(nc) as tc:
        with (
            tc.tile_pool(name="win", bufs=1) as wpool,
            tc.tile_pool(name="inp", bufs=in_bufs) as ipool,
            tc.tile_pool(name="outp", bufs=out_bufs) as opool,
            tc.tile_pool(name="ps", bufs=psum_bufs, space="PSUM") as ppool,
        ):
            cw_t = wpool.tile([128, lay["cw_pad"]], F32R, tag="consts")
            nc.sync.dma_start(out=cw_t[:], in_=consts_d[:])
            bias_t = cw_t[0:M, lay["off_bias"]:lay["off_bias"] + 1].bitcast(F32)
            if d_last:
                Ml = COUT * d_last
                bl_t = cw_t[0:Ml, lay["off_bl"]:lay["off_bl"] + 1].bitcast(F32)

            n_blocks = n_full + (1 if d_last else 0)
            for b in range(n_blocks):
                last = d_last and b == n_full
                Db = d_last if last else D
                Rb = Db + 2
                Mb = COUT * Db
                Kb = Rb * CIN

                t_full = ipool.tile([128, _pad32(W)], F32R, tag="inp")
                t = t_full[:, 0:W]
                # rolled input: row s of the rolled image = inp row (s-1)%H;
                # block b needs rolled rows [D*b, D*b+Rb) on partitions
                # p = q*CIN + c  (q = row-in-block, c = cin)
                r0 = D * b - 1
                if b == 0:
                    nc.sync.dma_start(
                        out=t[CIN:Rb * CIN, :],
                        in_=inp_d[:, 0:Rb - 1, :].rearrange("c q w -> q c w"))
                    nc.sync.dma_start(
                        out=t[0:CIN, :],
                        in_=inp_d[:, H - 1:H, :].rearrange("c q w -> q c w"))
                else:
                    nc.sync.dma_start(
                        out=t[0:Rb * CIN, :],
                        in_=inp_d[:, r0:r0 + Rb, :].rearrange("c q w -> q c w"))

                ot_full = opool.tile([M, _pad32(WOUT)], F32, tag="outp")
                ot = ot_full[:, 0:WOUT]
                for (c0, n) in chunks:
                    ps = ppool.tile([Mb, n], F32, tag="ps")
                    for j in range(KW):
                        if last:
                            lhsT = cw_t[0:Kb,
                                        lay["off_wl"] + j * Mb:
                                        lay["off_wl"] + (j + 1) * Mb]
                        else:
                            lhsT = cw_t[:, j * M:(j + 1) * M]
                        nc.tensor.matmul(
                            ps[:],
                            lhsT=lhsT,
                            rhs=t[0:Kb, c0 + j:c0 + j + n],
                            start=(j == 0),
                            stop=(j == KW - 1),
                        )
                    nc.vector.tensor_scalar_add(
                        ot[0:Mb, c0:c0 + n], ps[:],
                        (bl_t if last else bias_t))
                nc.scalar.dma_start(
                    out=out_d[:, D * b:D * b + Db, :].rearrange(
                        "co x w -> x co w"),
                    in_=ot[0:Mb, :])

    nc.compile()
    meta = dict(H=H, W=W, D=D, HOUT=HOUT, WOUT=WOUT, d_last=d_last, lay=lay)
    return nc, meta


def _fill_wmat(wmat, filt, D, col0):
    """wmat[q*CIN+c, col0 + j*COUT*D + co*D + dx] = filt[co, c, q-dx, j]."""
    Md = COUT * D
    for j in range(KW):
        for q in range(D + 2):
            for dx in range(D):
                i = q - dx
                if 0 <= i < KH:
                    for c in range(CIN):
                        wmat[q * CIN + c,
                             col0 + j * Md + dx * COUT + np.arange(COUT)] = \
                            filt[:, c, i, j]


def make_consts(filt: np.ndarray, bias: np.ndarray, D: int, d_last: int):
    """Host-side prep of filter+bias into the packed SBUF consts layout."""
    lay = _const_layout(D, d_last)
    consts = np.zeros((128, lay["cw_pad"]), np.float32)
    _fill_wmat(consts, filt, D, 0)
    consts[0:COUT * D, lay["off_bias"]] = np.tile(bias, D)
    if d_last:
        _fill_wmat(consts, filt, d_last, lay["off_wl"])
        consts[0:COUT * d_last, lay["off_bl"]] = np.tile(bias, d_last)
    return consts


_CACHE = {}


def _get_nc():
    if "nc" not in _CACHE:
        _CACHE["nc"] = build_nc()
    return _CACHE["nc"]


def kernel(inp: np.ndarray, filt: np.ndarray, bias: np.ndarray) -> np.ndarray:
    inp = np.asarray(inp, np.float32)
    filt = np.asarray(filt, np.float32)
    bias = np.asarray(bias, np.float32)
    nc, meta = _get_nc()
    consts = make_consts(filt, bias, meta["D"], meta["d_last"])
    in_maps = [
        {"inp": np.ascontiguousarray(inp[n]), "consts": consts}
        for n in range(N_CORES)
    ]
    res = run_bass_kernel_spmd(nc, in_maps, list(range(N_CORES)))
    out = np.stack([res.results[c]["out"] for c in range(N_CORES)], axis=0)
    return out



# revision 2
# speedup vs baseline: 4.2898x; 4.2898x over previous
"""Trainium2 Bass kernel: Conv2d [8,8,1024,1024] x [8,8,3,3] (+bias), with
the reference's roll-by-1 on H, VALID padding -> [8,8,1022,1022].

Strategy: data-parallel over the batch dim (1 image per NeuronCore, 8 cores).
The kernel is HBM-bandwidth bound (and the device HAM throttles HBM to ~50%
when all 8 cores stream), so the wire format is bf16 both ways and the host
pre-relayouts the input so every DMA moves big contiguous per-partition
spans:

  - Host packs the (rolled) input as inp_re[q*8+c, b*1024+w] =
    bf16(inp[c, (14b+q-1)%1024, w]): partition p = (row-in-block q, cin c),
    one 1024-col slab per conv block b (73 blocks x 14 output rows = 1022).
  - Per group of G=8..9 blocks, ONE input DMA moves [128, G*1024] with
    G*2KiB contiguous per partition; compute runs blocked matmuls on the
    tensor engine: lhsT [128,112] bf16 packs filt taps (column m = dx*8+co
    holds filt[co,c,q-dx,j]); the 3 W-taps are accumulating matmuls whose
    rhs is the same tile shifted by j. PSUM f32 [112,512] is evicted by
    DVE tensor_scalar_add(+bias) into a bf16 SBUF tile, and ONE output DMA
    per group writes [112, G*1022] contiguous.
  - Host unshards out_re[dx*8+co, b*1022+w] -> out[co, 14b+dx, w] and
    upcasts to f32.

bf16 error (inputs+weights+output quantized, f32 PSUM accumulate over the
72-term contraction) is ~0.3% of output scale, well under the 2e-2 gate.
"""

import os
import sys

for _p in ("/opt/trn_rl_repo",):
    if _p not in sys.path and os.path.isdir(_p):
        sys.path.insert(0, _p)

import ml_dtypes
import numpy as np

import concourse.bacc as bacc
import concourse.mybir as mybir
from concourse.bass_utils import run_bass_kernel_spmd
from concourse.tile import TileContext

F32 = mybir.dt.float32
BF16 = mybir.dt.bfloat16
NPBF16 = ml_dtypes.bfloat16

N_CORES = 8
CIN = 8
COUT = 8
KH = 3
KW = 3
H = W = 1024
HOUT = WOUT = 1022
D = 14            # output rows per block
R = D + 2         # input rows per block
M = COUT * D      # 112 matmul output columns (dx-major)
NB = HOUT // D    # 73 blocks exactly
GROUPS = [8] * 8 + [9]   # sum = 73
GMAX = max(GROUPS)
CHUNKS = ((0, 512), (512, 510))  # PSUM bank = 512 f32


def build_nc(in_bufs: int = 3, out_bufs: int = 3, psum_bufs: int = 8):
    nc = bacc.Bacc("TRN2", target_bir_lowering=False, debug=False,
                   num_devices=N_CORES)
    inp_d = nc.dram_tensor("inp", [R * CIN, NB * W], BF16,
                           kind="ExternalInput")
    wgt_d = nc.dram_tensor("wgt", [R * CIN, KW * M], BF16,
                           kind="ExternalInput")
    bias_d = nc.dram_tensor("bias", [M, 1], F32, kind="ExternalInput")
    out_d = nc.dram_tensor("out", [M, NB * WOUT], BF16,
                           kind="ExternalOutput")

    with TileContext(nc) as tc:
        with (
            tc.tile_pool(name="consts", bufs=1) as wpool,
            tc.tile_pool(name="inp", bufs=in_bufs) as ipool,
            tc.tile_pool(name="outp", bufs=out_bufs) as opool,
            tc.tile_pool(name="ps", bufs=psum_bufs, space="PSUM") as ppool,
        ):
            wt = wpool.tile([R * CIN, KW * M], BF16, tag="wgt")
            nc.sync.dma_start(out=wt[:], in_=wgt_d[:])
            bt = wpool.tile([M, 1], F32, tag="bias")
            nc.sync.dma_start(out=bt[:], in_=bias_d[:])

            b0 = 0
            for G in GROUPS:
                t = ipool.tile([R * CIN, GMAX * W], BF16, tag="inp")
                nc.sync.dma_start(out=t[:, 0:G * W],
                                  in_=inp_d[:, b0 * W:(b0 + G) * W])
                ot = opool.tile([M, GMAX * WOUT], BF16, tag="outp")
                for bl in range(G):
                    for (c0, n) in CHUNKS:
                        ps = ppool.tile([M, 512], F32, tag="ps")
                        for j in range(KW):
                            nc.tensor.matmul(
                                ps[:, 0:n],
                                lhsT=wt[:, j * M:(j + 1) * M],
                                rhs=t[:, bl * W + c0 + j:bl * W + c0 + j + n],
                                start=(j == 0),
                                stop=(j == KW - 1),
                            )
                        nc.vector.tensor_scalar_add(
                            ot[:, bl * WOUT + c0:bl * WOUT + c0 + n],
                            ps[:, 0:n], bt)
                nc.scalar.dma_start(out=out_d[:, b0 * WOUT:(b0 + G) * WOUT],
                                    in_=ot[:, 0:G * WOUT])
                b0 += G

    nc.compile()
    return nc


def _relayout_input(x):
    """[CIN,H,W] f32 -> [128, NB*W] bf16 with the roll + halo baked in."""
    xb = x.astype(NPBF16)
    rows = (D * np.arange(NB)[:, None] + np.arange(R)[None, :] - 1) % H
    g = xb[:, rows, :]                      # [c, b, q, w]
    return np.ascontiguousarray(g.transpose(2, 0, 1, 3)).reshape(
        R * CIN, NB * W)


def _pack_weights(filt):
    """wgt[q*CIN+c, j*M + dx*COUT + co] = filt[co, c, q-dx, j]."""
    wm = np.zeros((R * CIN, KW * M), np.float32)
    for j in range(KW):
        for q in range(R):
            for dx in range(D):
                i = q - dx
                if 0 <= i < KH:
                    for c in range(CIN):
                        wm[q * CIN + c, j * M + dx * COUT:
                           j * M + dx * COUT + COUT] = filt[:, c, i, j]
    return wm.astype(NPBF16)


def _prep_in_maps(inp, filt, bias):
    inp = np.asarray(inp, np.float32)
    filt = np.asarray(filt, np.float32)
    bias = np.asarray(bias, np.float32)
    wgt = _pack_weights(filt)
    bias112 = np.ascontiguousarray(np.tile(bias, D)[:, None])
    return [
        {"inp": _relayout_input(inp[n]), "wgt": wgt, "bias": bias112}
        for n in range(N_CORES)
    ]


def _unshard(res):
    outs = []
    for c in range(N_CORES):
        o = np.asarray(res.results[c]["out"]).astype(np.float32)
        o = o.reshape(D, COUT, NB, WOUT).transpose(1, 2, 0, 3)
        outs.append(o.reshape(COUT, HOUT, WOUT))
    return np.stack(outs, axis=0)


_CACHE = {}


def _get_nc():
    if "nc" not in _CACHE:
        _CACHE["nc"] = build_nc()
    return _CACHE["nc"]


def kernel(inp: np.ndarray, filt: np.ndarray, bias: np.ndarray) -> np.ndarray:
    nc = _get_nc()
    in_maps = _prep_in_maps(inp, filt, bias)
    res = run_bass_kernel_spmd(nc, in_maps, list(range(N_CORES)))
    return _unshard(res)


# revision 4
# speedup vs baseline: 4.5631x; 1.0637x over previous
"""Trainium2 Bass kernel: Conv2d [8,8,1024,1024] x [8,8,3,3] (+bias), with
the reference's roll-by-1 on H, VALID padding -> [8,8,1022,1022].

Strategy: data-parallel over the batch dim (1 image per NeuronCore, 8 cores).
The kernel is HBM-bandwidth bound (and the device HAM throttles HBM to ~50%
when all 8 cores stream), so the wire format is bf16 both ways and the host
pre-relayouts the input so every DMA moves big contiguous per-partition
spans:

  - Host packs the (rolled) input as inp_re[q*8+c, b*1024+w] =
    bf16(inp[c, (14b+q-1)%1024, w]): partition p = (row-in-block q, cin c),
    one 1024-col slab per conv block b (73 blocks x 14 output rows = 1022).
  - Per group of G=8..9 blocks, ONE input DMA moves [128, G*1024] with
    G*2KiB contiguous per partition; compute runs blocked matmuls on the
    tensor engine: lhsT [128,112] bf16 packs filt taps (column m = dx*8+co
    holds filt[co,c,q-dx,j]); the 3 W-taps are accumulating matmuls whose
    rhs is the same tile shifted by j. PSUM f32 [112,512] is evicted by
    DVE tensor_scalar_add(+bias) into a bf16 SBUF tile, and ONE output DMA
    per group writes [112, G*1022] contiguous.
  - Host unshards out_re[dx*8+co, b*1022+w] -> out[co, 14b+dx, w] and
    upcasts to f32.

bf16 error (inputs+weights+output quantized, f32 PSUM accumulate over the
72-term contraction) is ~0.3% of output scale, well under the 2e-2 gate.
"""

import os
import sys

for _p in ("/opt/trn_rl_repo",):
    if _p not in sys.path and os.path.isdir(_p):
        sys.path.insert(0, _p)

import ml_dtypes
import numpy as np

import concourse.bacc as bacc
import concourse.mybir as mybir
from concourse.bass_utils import run_bass_kernel_spmd
from concourse.tile import TileContext

F32 = mybir.dt.float32
BF16 = mybir.dt.bfloat16
NPBF16 = ml_dtypes.bfloat16

N_CORES = 8
CIN = 8
COUT = 8
KH = 3
KW = 3
H = W = 1024
HOUT = WOUT = 1022
D = 14            # output rows per block
R = D + 2         # input rows per block
M = COUT * D      # 112 matmul output columns (dx-major)
NB = HOUT // D    # 73 blocks exactly
GROUPS = [2, 3] + [4] * 17   # sum = 73; small head for fast pipeline fill
GMAX = max(GROUPS)
CHUNKS = ((0, 512), (512, 510))  # PSUM bank = 512 f32


def build_nc(in_bufs: int = 6, out_bufs: int = 6, psum_bufs: int = 8):
    nc = bacc.Bacc("TRN2", target_bir_lowering=False, debug=False,
                   num_devices=N_CORES)
    inp_d = nc.dram_tensor("inp", [R * CIN, NB * W], BF16,
                           kind="ExternalInput")
    wgt_d = nc.dram_tensor("wgt", [R * CIN, KW * M], BF16,
                           kind="ExternalInput")
    bias_d = nc.dram_tensor("bias", [M, 1], F32, kind="ExternalInput")
    out_d = nc.dram_tensor("out", [M, NB * WOUT], BF16,
                           kind="ExternalOutput")

    with TileContext(nc) as tc:
        with (
            tc.tile_pool(name="consts", bufs=1) as wpool,
            tc.tile_pool(name="inp", bufs=in_bufs) as ipool,
            tc.tile_pool(name="outp", bufs=out_bufs) as opool,
            tc.tile_pool(name="ps", bufs=psum_bufs, space="PSUM") as ppool,
        ):
            ident = mybir.ActivationFunctionType.Identity
            wt = wpool.tile([R * CIN, KW * M], BF16, tag="wgt")
            nc.sync.dma_start(out=wt[:], in_=wgt_d[:])
            bt = wpool.tile([M, 1], F32, tag="bias")
            nc.sync.dma_start(out=bt[:], in_=bias_d[:])

            b0 = 0
            for G in GROUPS:
                t = ipool.tile([R * CIN, GMAX * W], BF16, tag="inp")
                nc.sync.dma_start(out=t[:, 0:G * W],
                                  in_=inp_d[:, b0 * W:(b0 + G) * W])
                ot = opool.tile([M, GMAX * WOUT], BF16, tag="outp")
                for bl in range(G):
                    for ci, (c0, n) in enumerate(CHUNKS):
                        ps = ppool.tile([M, 512], F32, tag="ps")
                        for j in range(KW):
                            nc.tensor.matmul(
                                ps[:, 0:n],
                                lhsT=wt[:, j * M:(j + 1) * M],
                                rhs=t[:, bl * W + c0 + j:bl * W + c0 + j + n],
                                start=(j == 0),
                                stop=(j == KW - 1),
                            )
                        # Evict PSUM(+bias) on alternating engines: the
                        # f32 PSUM read is the per-engine throughput wall.
                        dst = ot[:, bl * WOUT + c0:bl * WOUT + c0 + n]
                        if ci == 0:
                            nc.vector.tensor_scalar_add(dst, ps[:, 0:n], bt)
                        else:
                            nc.scalar.activation(dst, ps[:, 0:n], ident,
                                                 bias=bt)
                nc.gpsimd.dma_start(out=out_d[:, b0 * WOUT:(b0 + G) * WOUT],
                                    in_=ot[:, 0:G * WOUT])
                b0 += G

    nc.compile()
    return nc


def _relayout_input(x):
    """[CIN,H,W] f32 -> [128, NB*W] bf16 with the roll + halo baked in."""
    xb = x.astype(NPBF16)
    rows = (D * np.arange(NB)[:, None] + np.arange(R)[None, :] - 1) % H
    g = xb[:, rows, :]                      # [c, b, q, w]
    return np.ascontiguousarray(g.transpose(2, 0, 1, 3)).reshape(
        R * CIN, NB * W)


def _pack_weights(filt):
    """wgt[q*CIN+c, j*M + dx*COUT + co] = filt[co, c, q-dx, j]."""
    wm = np.zeros((R * CIN, KW * M), np.float32)
    for j in range(KW):
        for q in range(R):
            for dx in range(D):
                i = q - dx
                if 0 <= i < KH:
                    for c in range(CIN):
                        wm[q * CIN + c, j * M + dx * COUT:
                           j * M + dx * COUT + COUT] = filt[:, c, i, j]
    return wm.astype(NPBF16)


def _prep_in_maps(inp, filt, bias):
    inp = np.asarray(inp, np.float32)
    filt = np.asarray(filt, np.float32)
    bias = np.asarray(bias, np.float32)
    wgt = _pack_weights(filt)
    bias112 = np.ascontiguousarray(np.tile(bias, D)[:, None])
    return [
        {"inp": _relayout_input(inp[n]), "wgt": wgt, "bias": bias112}
        for n in range(N_CORES)
    ]


def _unshard(res):
    outs = []
    for c in range(N_CORES):
        o = np.asarray(res.results[c]["out"]).astype(np.float32)
        o = o.reshape(D, COUT, NB, WOUT).transpose(1, 2, 0, 3)
        outs.append(o.reshape(COUT, HOUT, WOUT))
    return np.stack(outs, axis=0)


_CACHE = {}


def _get_nc():
    if "nc" not in _CACHE:
        _CACHE["nc"] = build_nc()
    return _CACHE["nc"]


def kernel(inp: np.ndarray, filt: np.ndarray, bias: np.ndarray) -> np.ndarray:
    nc = _get_nc()
    in_maps = _prep_in_maps(inp, filt, bias)
    res = run_bass_kernel_spmd(nc, in_maps, list(range(N_CORES)))
    return _unshard(res)


# revision 9
# speedup vs baseline: 4.6046x; 1.0091x over previous
"""Trainium2 Bass kernel: Conv2d [8,8,1024,1024] x [8,8,3,3] (+bias), with
the reference's roll-by-1 on H, VALID padding -> [8,8,1022,1022].

Strategy: data-parallel over the batch dim (1 image per NeuronCore, 8 cores).
The kernel is HBM-bandwidth bound (and the device HAM throttles HBM to ~50%
when all 8 cores stream), so the wire format is bf16 both ways and the host
pre-relayouts the input so every DMA moves big contiguous per-partition
spans:

  - Host packs the (rolled) input as inp_re[q*8+c, b*1024+w] =
    bf16(inp[c, (14b+q-1)%1024, w]): partition p = (row-in-block q, cin c),
    one 1024-col slab per conv block b (73 blocks x 14 output rows = 1022).
  - Per group of G=8..9 blocks, ONE input DMA moves [128, G*1024] with
    G*2KiB contiguous per partition; compute runs blocked matmuls on the
    tensor engine: lhsT [128,112] bf16 packs filt taps (column m = dx*8+co
    holds filt[co,c,q-dx,j]); the 3 W-taps are accumulating matmuls whose
    rhs is the same tile shifted by j. PSUM f32 [112,512] is evicted by
    DVE tensor_scalar_add(+bias) into a bf16 SBUF tile, and ONE output DMA
    per group writes [112, G*1022] contiguous.
  - Host unshards out_re[dx*8+co, b*1022+w] -> out[co, 14b+dx, w] and
    upcasts to f32.

bf16 error (inputs+weights+output quantized, f32 PSUM accumulate over the
72-term contraction) is ~0.3% of output scale, well under the 2e-2 gate.
"""

import os
import sys

for _p in ("/opt/trn_rl_repo",):
    if _p not in sys.path and os.path.isdir(_p):
        sys.path.insert(0, _p)

import ml_dtypes
import numpy as np

import concourse.bacc as bacc
import concourse.mybir as mybir
from concourse.bass_utils import run_bass_kernel_spmd
from concourse.tile import TileContext

F32 = mybir.dt.float32
BF16 = mybir.dt.bfloat16
NPBF16 = ml_dtypes.bfloat16

N_CORES = 8
CIN = 8
COUT = 8
KH = 3
KW = 3
H = W = 1024
HOUT = WOUT = 1022
D = 14            # output rows per block
R = D + 2         # input rows per block
M = COUT * D      # 112 matmul output columns (dx-major)
MPAD = 128        # lhsT padded to 128 cols: enables PE Fast Weight Load
NB = HOUT // D    # 73 blocks exactly
GROUPS = [2, 3] + [4] * 16 + [3, 1]   # sum = 73; small head + tail
GMAX = max(GROUPS)
CHUNKS = ((0, 512), (512, 510))  # PSUM bank = 512 f32


def build_nc(in_bufs: int = 6, out_bufs: int = 6, psum_bufs: int = 8):
    nc = bacc.Bacc("TRN2", target_bir_lowering=False, debug=False,
                   num_devices=N_CORES)
    inp_d = nc.dram_tensor("inp", [R * CIN, NB * W], BF16,
                           kind="ExternalInput")
    wgt_d = nc.dram_tensor("wgt", [R * CIN, KW * MPAD], BF16,
                           kind="ExternalInput")
    bias_d = nc.dram_tensor("bias", [M, 1], F32, kind="ExternalInput")
    out_d = nc.dram_tensor("out", [M, NB * WOUT], BF16,
                           kind="ExternalOutput")

    with TileContext(nc) as tc:
        with (
            tc.tile_pool(name="consts", bufs=1) as wpool,
            tc.tile_pool(name="inp", bufs=in_bufs) as ipool,
            tc.tile_pool(name="outp", bufs=out_bufs) as opool,
            tc.tile_pool(name="ps", bufs=psum_bufs, space="PSUM") as ppool,
        ):
            ident = mybir.ActivationFunctionType.Identity
            wt = wpool.tile([R * CIN, KW * MPAD], BF16, tag="wgt")
            nc.sync.dma_start(out=wt[:], in_=wgt_d[:])
            bt = wpool.tile([M, 1], F32, tag="bias")
            nc.sync.dma_start(out=bt[:], in_=bias_d[:])

            b0 = 0
            for G in GROUPS:
                t = ipool.tile([R * CIN, GMAX * W], BF16, tag="inp")
                nc.sync.dma_start(out=t[:, 0:G * W],
                                  in_=inp_d[:, b0 * W:(b0 + G) * W])
                ot = opool.tile([M, GMAX * WOUT], BF16, tag="outp")
                for bl in range(G):
                    for ci, (c0, n) in enumerate(CHUNKS):
                        ps = ppool.tile([MPAD, 512], F32, tag="ps")
                        for j in range(KW):
                            nc.tensor.matmul(
                                ps[:, 0:n],
                                lhsT=wt[:, j * MPAD:(j + 1) * MPAD],
                                rhs=t[:, bl * W + c0 + j:bl * W + c0 + j + n],
                                start=(j == 0),
                                stop=(j == KW - 1),
                            )
                        # Evict PSUM(+bias) on alternating engines: the
                        # f32 PSUM read is the per-engine throughput wall.
                        dst = ot[:, bl * WOUT + c0:bl * WOUT + c0 + n]
                        if ci == 0:
                            nc.vector.tensor_scalar_add(dst, ps[0:M, 0:n], bt)
                        else:
                            nc.scalar.activation(dst, ps[0:M, 0:n], ident,
                                                 bias=bt)
                nc.gpsimd.dma_start(out=out_d[:, b0 * WOUT:(b0 + G) * WOUT],
                                    in_=ot[:, 0:G * WOUT])
                b0 += G

    nc.compile()
    return nc


def _relayout_input(x):
    """[CIN,H,W] f32 -> [128, NB*W] bf16 with the roll + halo baked in."""
    xb = x.astype(NPBF16)
    rows = (D * np.arange(NB)[:, None] + np.arange(R)[None, :] - 1) % H
    g = xb[:, rows, :]                      # [c, b, q, w]
    return np.ascontiguousarray(g.transpose(2, 0, 1, 3)).reshape(
        R * CIN, NB * W)


def _pack_weights(filt):
    """wgt[q*CIN+c, j*MPAD + dx*COUT + co] = filt[co, c, q-dx, j]."""
    wm = np.zeros((R * CIN, KW * MPAD), np.float32)
    for j in range(KW):
        for q in range(R):
            for dx in range(D):
                i = q - dx
                if 0 <= i < KH:
                    for c in range(CIN):
                        wm[q * CIN + c, j * MPAD + dx * COUT:
                           j * MPAD + dx * COUT + COUT] = filt[:, c, i, j]
    return wm.astype(NPBF16)


def _prep_in_maps(inp, filt, bias):
    inp = np.asarray(inp, np.float32)
    filt = np.asarray(filt, np.float32)
    bias = np.asarray(bias, np.float32)
    wgt = _pack_weights(filt)
    bias112 = np.ascontiguousarray(np.tile(bias, D)[:, None])
    return [
        {"inp": _relayout_input(inp[n]), "wgt": wgt, "bias": bias112}
        for n in range(N_CORES)
    ]


def _unshard(res):
    outs = []
    for c in range(N_CORES):
        o = np.asarray(res.results[c]["out"]).astype(np.float32)
        o = o.reshape(D, COUT, NB, WOUT).transpose(1, 2, 0, 3)
        outs.append(o.reshape(COUT, HOUT, WOUT))
    return np.stack(outs, axis=0)


_CACHE = {}


def _get_nc():
    if "nc" not in _CACHE:
        _CACHE["nc"] = build_nc()
    return _CACHE["nc"]


def kernel(inp: np.ndarray, filt: np.ndarray, bias: np.ndarray) -> np.ndarray:
    nc = _get_nc()
    in_maps = _prep_in_maps(inp, filt, bias)
    res = run_bass_kernel_spmd(nc, in_maps, list(range(N_CORES)))
    return _unshard(res)
